# revision 7
# baseline (speedup 1.0000x reference)
"""Trainium2 Bass kernel for nn_CoefficientDecoder — transposed/row-tiled.

reference:  h = relu(x @ W1.T + b1); three 64x64 linears; z = h @ W2.T + b2;
            out = z @ bases.  Everything after the ReLU is linear, so the
whole tail constant-folds host-side (x-independent, float64):
    h   = relu(x @ W1.T + b1)            [B, 64]
    out = h @ Beff + beff                Beff = (W2@Wd1@Wd2@Wd3).T @ bases
This cuts device FLOPs 8x (contraction 512 -> 64) and input HBM traffic to
~1 MB/core; fp16 operands + fp16 output halve the store traffic (rel-err
lands ~3e-4 against the 2e-2 gate).  Data-parallel over batch: 8 cores x
1024 rows, weights replicated.

This variant computes out TRANSPOSED (outT[4096, 1024] per core) so the
seq-dim bias beff is per-PARTITION: it fuses into the PSUM->SBUF copy on
both engines (ACT Identity+bias, DVE tensor_scalar_add).  That frees the
contraction to be exactly K=64, enabling row-tiled CONCURRENT matmul
pairs: even seq-blocks use PE rows 0-63, odd blocks rows 64-127 (distinct
row groups run concurrently per the tensor-engine doc).  Both h and Beff
are duplicated across the two partition halves; h's duplication is free
(layer-1 stationary widened to [W1T | W1T] so one matmul writes h twice).

Per 128-row seq block sp: op[128,1024] psum = Beff_dup[64h:+64, sp*128:+128].T
@ h2[64h:+64, jb*512:+512] for jb in 0,1; bias+fp16 via DVE/ACT; staging
pairs (sp even+odd) ship as 512 KB DMA stores on alternating HWDGE queues.
Host transposes shards back at gather time.
"""

import numpy as np

import concourse.bass as bass
import concourse.tile as tile
from concourse import bacc, mybir
from concourse.bass import ts
from concourse.bass_utils import run_bass_kernel_spmd

N_CORES = 8
B, IN_F, HID, NB, SEQ = 8192, 256, 64, 512, 4096
B_LOC = B // N_CORES            # 1024 batch rows per core

F32 = mybir.dt.float32
F16 = mybir.dt.float16

OUT_MODE = "f16"

_CACHE = {}


def _build(out_mode: str, repeat: int = 1):
    out_dt = F16 if out_mode == "f16" else F32

    nc = bacc.Bacc(
        "TRN2",
        target_bir_lowering=False,
        debug=False,
        enable_asserts=False,
        num_devices=N_CORES,
    )

    xT_d = nc.declare_dram_parameter("xT", [IN_F, B_LOC], F16, isOutput=False)
    w1_d = nc.declare_dram_parameter("w1", [128, 2, 128], F16, isOutput=False)
    b1_d = nc.declare_dram_parameter("b1", [128, 1], F32, isOutput=False)
    beff_d = nc.declare_dram_parameter("beff", [128, SEQ], F16, isOutput=False)
    bv_d = nc.declare_dram_parameter("bv", [128, SEQ // 128], F32, isOutput=False)
    out_d = nc.declare_dram_parameter("out", [SEQ, B_LOC], out_dt, isOutput=True)

    KC = IN_F // 128        # 2 k-chunks for layer 1
    NJ = B_LOC // 512       # 2 batch chunks (moving dim)
    SP = SEQ // 128         # 32 seq blocks (outT partition blocks)
    SG = 2                  # seq blocks per store

    relu = mybir.ActivationFunctionType.Relu
    ident = mybir.ActivationFunctionType.Identity

    with tile.TileContext(nc) as tc:
        with (
            tc.tile_pool(name="const", bufs=1) as constp,
            tc.tile_pool(name="xsb", bufs=1) as xsbp,
            tc.tile_pool(name="ht", bufs=1) as htp,
            tc.tile_pool(name="outsb", bufs=4) as outsbp,
            tc.tile_pool(name="mlp_ps", bufs=2, space="PSUM") as mlpp,
            tc.tile_pool(name="out_ps", bufs=3, space="PSUM") as outpp,
        ):
            def body():
                xT_pkn = xT_d.rearrange("(k p) n -> p k n", p=128)
                out_pgn = out_d.rearrange("(g p) n -> p g n", p=128)
                xsb = xsbp.tile([128, KC, B_LOC], F16, tag="x")
                w1 = constp.tile([128, KC, 128], F16, tag="w1")
                b1 = constp.tile([128, 1], F32, tag="b1")
                beff = constp.tile([128, SEQ], F16, tag="beff")
                bv = constp.tile([128, SP], F32, tag="bv")
                h2 = htp.tile([128, B_LOC], F16, tag="h2")

                nc.sync.dma_start(xsb[:, 0, :], xT_pkn[:, 0, :])
                nc.scalar.dma_start(w1[:], w1_d[:])
                nc.scalar.dma_start(b1[:], b1_d[:])
                nc.sync.dma_start(xsb[:, 1, :], xT_pkn[:, 1, :])
                nc.scalar.dma_start(bv[:], bv_d[:])
                nc.sync.dma_start(beff[:], beff_d[:])

                # layer 1: one matmul per (j,k); stationary [W1T|W1T] writes
                # h duplicated across both partition halves for free
                for j in range(NJ):
                    hp = mlpp.tile([128, 512], F32, tag="mlp")
                    for k in range(KC):
                        nc.tensor.matmul(
                            hp[:], w1[:, k, :], xsb[:, k, ts(j, 512)],
                            start=(k == 0), stop=(k == KC - 1),
                        )
                    nc.scalar.activation(h2[:, ts(j, 512)], hp[:], relu, bias=b1)

                # main GEMM, outT blocks of 128 seq rows; even/odd sp on
                # distinct PE row groups -> concurrent matmuls
                for g in range(SP // SG):
                    ob = outsbp.tile([128, SG, B_LOC], out_dt, tag="ob")
                    for i in range(SG):
                        sp = g * SG + i
                        half = sp % 2
                        rows = slice(64 * half, 64 * half + 64)
                        op = outpp.tile([128, 1024], F32, tag="op")
                        for jb in range(NJ):
                            nc.tensor.matmul(
                                op[:, ts(jb, 512)],
                                beff[rows, ts(sp, 128)],
                                h2[rows, ts(jb, 512)],
                                start=True, stop=True,
                            )
                        if sp % 2 == 0:
                            nc.vector.tensor_scalar_add(
                                ob[:, i, :], op[:], bv[:, sp : sp + 1],
                            )
                        else:
                            nc.scalar.activation(
                                ob[:, i, :], op[:], ident, bias=bv[:, sp : sp + 1],
                            )
                    dma_eng = nc.sync if (g % 2 == 0) else nc.scalar
                    dma_eng.dma_start(out_pgn[:, ts(g, SG), :], ob[:])

            if repeat == 1:
                body()
            else:
                with tc.For_i(0, repeat, 1):
                    body()

    nc.compile()
    return nc


def _get_nc(out_mode: str, repeat: int = 1):
    key = (out_mode, repeat)
    if key not in _CACHE:
        _CACHE[key] = _build(out_mode, repeat)
    return _CACHE[key]


def _fold(W1, b1, Wd1, bd1, Wd2, bd2, Wd3, bd3, W2, b2, bases):
    """Constant-fold the linear tail (x-independent) in float64."""
    W1, b1, Wd1, bd1, Wd2, bd2, Wd3, bd3, W2, b2, bases = (
        np.asarray(a, np.float64)
        for a in (W1, b1, Wd1, bd1, Wd2, bd2, Wd3, bd3, W2, b2, bases)
    )
    W2eff = W2 @ Wd1 @ Wd2 @ Wd3                       # [512, 64]
    b2eff = b2 + (bd3 @ Wd2.T @ Wd1.T + bd2 @ Wd1.T + bd1) @ W2.T
    Beff = (W2eff.T @ bases).astype(np.float16)        # [64, 4096]
    bvec = (b2eff @ bases).astype(np.float32)          # [4096]
    return Beff, bvec


def _in_maps(x, W1, b1, Wd1, bd1, Wd2, bd2, Wd3, bd3, W2, b2, bases):
    Beff, bvec = _fold(W1, b1, Wd1, bd1, Wd2, bd2, Wd3, bd3, W2, b2, bases)
    W1T = np.asarray(W1, np.float32).T.astype(np.float16)    # [256, 64]
    w1c = W1T.reshape(2, 128, HID)
    common = {
        # stationary [W1T | W1T]: h lands duplicated on both halves
        "w1": np.ascontiguousarray(np.concatenate([w1c, w1c], axis=2)
                                   .transpose(1, 0, 2)),      # [128, 2, 128]
        "b1": np.ascontiguousarray(
            np.tile(np.asarray(b1, np.float32), 2).reshape(128, 1)),
        "beff": np.ascontiguousarray(np.concatenate([Beff, Beff], axis=0)),
        "bv": np.ascontiguousarray(bvec.reshape(SEQ // 128, 128).T),
    }
    xT = np.asarray(x, np.float32).T.astype(np.float16)      # [256, B]
    maps = []
    for i in range(N_CORES):
        m = dict(common)
        m["xT"] = np.ascontiguousarray(xT[:, i * B_LOC : (i + 1) * B_LOC])
        maps.append(m)
    return maps


def run(inputs: dict, out_mode: str = OUT_MODE, repeat: int = 1, **run_kwargs):
    """Shard, execute on 8 cores, gather (transposing back). Returns (out, res)."""
    nc = _get_nc(out_mode, repeat)
    in_maps = _in_maps(**{k: np.asarray(v) for k, v in inputs.items()})
    res = run_bass_kernel_spmd(nc, in_maps, list(range(N_CORES)), **run_kwargs)
    out = np.empty((B, SEQ), np.float32)
    for i in range(N_CORES):
        out[i * B_LOC : (i + 1) * B_LOC] = (
            np.asarray(res.results[i]["out"]).T.astype(np.float32)
        )
    return out, res


def kernel(**inputs) -> np.ndarray:
    out, _ = run(inputs)
    return out
